# revision 30
# baseline (speedup 1.0000x reference)
"""SigLIP loss kernel for 8 Trainium2 NeuronCores.

Strategy:
  - Row-shard video_embed across the 8 cores (1024 rows each); every core
    reads the full text matrix from its own HBM (cheaper than ring-exchange
    over the inter-core links, whose effective bandwidth is far below HBM).
  - text_embed is laid out [D, N] by the host (pure permutation; all
    arithmetic stays on device), so the contraction dim lands on SBUF
    partitions without any on-device transposes. The l2-normalization is a
    VectorEngine broadcast multiply with host-computed inverse norms (the
    logit scale is split geometrically between the two operands to center
    both in fp8e4m3's dynamic range).
  - Logits: fp8e4m3 matmuls in DoubleRow mode (K=256 per instruction),
    fp32 PSUM accumulation, 2048-wide PSUM groups so one LDWEIGHTS serves
    four matmuls (plus an IR pass that elides the redundant reloads).
  - softplus(x) = ln(exp(x)+1) on the ScalarEngine (this toolchain has no
    softplus table; Exp+Ln share one table set) with the per-row loss sum
    accumulated for free via the activation accumulator; the bf16 exp
    scratch doubles as the row-max source (exp is monotone).
  - Host: inverse norms + exp of the scale (O(N) prep), final scalar
    reduction, and an exact float64 recheck of rows whose diag-vs-max
    margin falls inside the fp8 error band — the argmax accuracy is exact.
"""

from contextlib import ExitStack

import numpy as np

N, D = 8192, 768
P = 128
KC = D // P            # 6 contraction chunks of 128
NCORES = 8
NV = N // NCORES       # 1024 v rows per core
NVB = NV // P          # 8 v blocks of 128 rows
TBW = 512              # matmul moving free dim (ISA max)
QW = 4                 # 512-col quarters per PSUM group (2048 cols)
CB = QW * TBW          # 2048-column blocks
NT = N // CB           # 4 outer column blocks
# fp8e4m3 inputs give per-logit error ~0.05 absolute; rows whose diag-vs-max
# margin lies inside this band are recomputed exactly on the host.
MARGIN_BAND = 0.3

_COMPILED = None


def _build_nc():
    import concourse.mybir as mybir
    import concourse.tile as tile
    from concourse import bacc
    from concourse.masks import make_identity

    f32 = mybir.dt.float32
    bf16 = mybir.dt.bfloat16
    fp8 = mybir.dt.float8e4
    DR = mybir.MatmulPerfMode.DoubleRow
    EXP = mybir.ActivationFunctionType.Exp
    LN = mybir.ActivationFunctionType.Ln
    AX = mybir.AxisListType.X
    AXY = mybir.AxisListType.XY

    nc = bacc.Bacc(
        "TRN2",
        target_bir_lowering=False,
        debug=False,
        enable_asserts=False,
        num_devices=NCORES,
    )

    # Make Exp and Ln resolve to the single table set containing both so one
    # ACT_TABLE_LOAD suffices (set order preserved — ids stay valid).
    orig_tables = dict(bacc.get_activation_tables(nc.m.arch))
    patched = {
        name: (fns if name == "natural_log_exp_and_others" else fns - {EXP, LN})
        for name, fns in orig_tables.items()
    }
    bacc.get_activation_tables = lambda arch: patched

    v_d = nc.dram_tensor("v", [NV, D], f32, kind="ExternalInput")
    tT_d = nc.dram_tensor("tT", [D, N], f32, kind="ExternalInput")
    invv_d = nc.dram_tensor("inv_v", [P, NVB], f32, kind="ExternalInput")
    invt_d = nc.dram_tensor("inv_t", [1, N], f32, kind="ExternalInput")
    rs_d = nc.dram_tensor("row_sum", [P, NVB], f32, kind="ExternalOutput")
    rm_d = nc.dram_tensor("row_max", [P, NVB], f32, kind="ExternalOutput")

    with tile.TileContext(nc) as tc, ExitStack() as ctx:
        singles = ctx.enter_context(tc.tile_pool(name="singles", bufs=1))
        tstage = ctx.enter_context(tc.tile_pool(name="tstage", bufs=3))
        ttp = ctx.enter_context(tc.tile_pool(name="ttp", bufs=3))
        spp = ctx.enter_context(tc.tile_pool(name="spp", bufs=3))
        psum_mm = ctx.enter_context(tc.tile_pool(name="psum_mm", bufs=2, space="PSUM"))

        ident = singles.tile([P, P], bf16)
        make_identity(nc, ident)

        invv = singles.tile([P, NVB], f32)
        nc.gpsimd.dma_start(out=invv, in_=invv_d.ap())
        # inverse text norms broadcast across all partitions (DMA replicates)
        invb = singles.tile([P, N], f32)
        nc.gpsimd.dma_start(out=invb, in_=invt_d.ap().to_broadcast([P, N]))

        rs_cols = singles.tile([P, NVB, NT], f32)
        mx_cols = singles.tile([P, NVB, NT], bf16)

        # ---- main loop over 2048-column blocks, software-pipelined: the
        # next block's loads and normalize-multiplies are emitted between
        # this block's matmul groups so every engine stays fed.
        blocks = {}

        def prep_dma(tb, half):
            tst = tstage.tile([P, KC, CB // 2], f32, tag="tst", name=f"tst{tb}_{half}")
            nc.gpsimd.dma_start(
                out=tst,
                in_=tT_d.ap().rearrange("(k p) c -> p k c", p=P)[
                    :, :, tb * CB + half * (CB // 2) : tb * CB + (half + 1) * (CB // 2)
                ],
            )
            if half == 0:
                blocks[tb] = [
                    tst,
                    None,
                    ttp.tile([P, KC, CB], fp8, tag="ttf", name=f"ttf{tb}"),
                ]
            else:
                blocks[tb][1] = tst

        def prep_mul(tb, half, k):
            """normalize one [128, 1024] slice into the fp8 matmul operand."""
            tst = blocks[tb][half]
            ttf = blocks[tb][2]
            c0 = tb * CB + half * (CB // 2)
            nc.gpsimd.tensor_mul(
                ttf[:, k, half * (CB // 2) : (half + 1) * (CB // 2)],
                tst[:, k, :],
                invb[:, c0 : c0 + CB // 2],
            )

        prep_dma(0, 0)
        prep_dma(0, 1)

        # ---- v prep: load, normalize+cast, PE-transpose to [d, row] fp8.
        # Emitted after the first text block's DMAs so the t pipeline starts
        # flowing while the video shard is prepared.
        vst = tstage.tile([P, NVB, D], f32, tag="tst", name="vst")
        nc.gpsimd.dma_start(out=vst, in_=v_d.ap().rearrange("(a p) d -> p a d", p=P))
        for k in range(KC):
            prep_mul(0, 0, k)
            prep_mul(0, 1, k)
        vbf = singles.tile([P, NVB, D], bf16)
        for vb in range(NVB):
            nc.vector.tensor_scalar_mul(
                vbf[:, vb, :], vst[:, vb, :], invv[:, vb : vb + 1]
            )
        vT = singles.tile([P, KC, NV], fp8)
        for vb in range(NVB):
            for k in range(KC):
                # prologue-only: borrow a psum_mm slot for the transposes
                pt = psum_mm.tile([P, P], bf16, tag="ps", name=f"pt{vb}_{k}")
                nc.tensor.transpose(pt, vbf[:, vb, k * P : (k + 1) * P], ident)
                nc.vector.tensor_copy(vT[:, k, vb * P : (vb + 1) * P], pt)

        for tb in range(NT):
            ttf = blocks.pop(tb)[2]
            if tb + 1 < NT:
                prep_dma(tb + 1, 0)
                prep_dma(tb + 1, 1)
            for vb in range(NVB):
                ps = psum_mm.tile([P, QW, TBW], f32, tag="ps", name=f"ps{tb}_{vb}")
                # kk outer / q inner: the four matmuls of one kk share lhsT,
                # so the duplicate-LDWEIGHTS pass drops 3 of 4 weight loads.
                for kk in range(KC // 2):
                    for q in range(QW):
                        nc.tensor.matmul(
                            ps[:, q, :],
                            vT[:, 2 * kk : 2 * kk + 2, vb * P : (vb + 1) * P],
                            ttf[:, 2 * kk : 2 * kk + 2, q * TBW : (q + 1) * TBW],
                            start=(kk == 0),
                            stop=(kk == KC // 2 - 1),
                            perf_mode=DR,
                        )
                # softplus(x) = ln(exp(x)+1); bf16 exp scratch doubles as the
                # row-max source (exp is monotone, host takes the log).
                ex = spp.tile([P, QW, TBW], bf16)
                nc.scalar.activation(ex, ps, EXP)
                sp = spp.tile([P, QW, TBW], bf16, tag="sp_dead")
                nc.scalar.activation(
                    sp, ex, LN, bias=1.0, accum_out=rs_cols[:, vb, tb : tb + 1]
                )
                nc.vector.tensor_reduce(
                    mx_cols[:, vb, tb : tb + 1],
                    ex,
                    axis=AXY,
                    op=mybir.AluOpType.max,
                )
                if tb + 1 < NT and vb < 6:
                    prep_mul(tb + 1, vb % 2, vb // 2 * 2)
                    prep_mul(tb + 1, vb % 2, vb // 2 * 2 + 1)

        rs_out = singles.tile([P, NVB], f32)
        mx_out = singles.tile([P, NVB], f32)
        for vb in range(NVB):
            nc.vector.reduce_sum(rs_out[:, vb : vb + 1], rs_cols[:, vb, :], axis=AX)
            nc.vector.reduce_max(mx_out[:, vb : vb + 1], mx_cols[:, vb, :], axis=AX)
        nc.sync.dma_start(out=rs_d.ap(), in_=rs_out)
        nc.sync.dma_start(out=rm_d.ap(), in_=mx_out)

    _elide_duplicate_ldweights(nc, mybir)
    nc.compile()
    return nc


def _elide_duplicate_ldweights(nc, mybir):
    """Drop an LDWEIGHTS that reloads the exact weights the PE already holds
    (sync-free and immediately consecutive in the PE program order)."""

    def _sig(ins):
        return repr(ins.ins[-1]), getattr(ins, "is_transpose", None)

    removed = 0
    for f in nc.m.functions:
        for bb in f.blocks:
            last_sig = None
            keep = []
            for ins in bb.instructions:
                eng = getattr(ins, "engine", None)
                if eng != mybir.EngineType.PE:
                    keep.append(ins)
                    continue
                if isinstance(ins, mybir.InstLdweights):
                    si = ins.sync_info
                    clean = si is None or (
                        len(si.on_wait) == 0 and len(si.on_update) == 0
                    )
                    sig = _sig(ins)
                    if clean and sig == last_sig:
                        removed += 1
                        continue
                    last_sig = sig
                    keep.append(ins)
                elif isinstance(ins, mybir.InstMatmult):
                    keep.append(ins)  # matmul does not disturb loaded weights
                else:
                    last_sig = None
                    keep.append(ins)
            bb.instructions = keep
    return removed


def _get_compiled():
    global _COMPILED
    if _COMPILED is None:
        _COMPILED = _build_nc()
    return _COMPILED


def _run_device(v32, tT32, inv_v, inv_t, trace=False):
    from concourse.bass_utils import run_bass_kernel_spmd

    nc = _get_compiled()
    in_maps = []
    for c in range(NCORES):
        sl = slice(c * NV, (c + 1) * NV)
        in_maps.append(
            {
                "v": np.ascontiguousarray(v32[sl]),
                "tT": tT32,
                "inv_v": np.ascontiguousarray(
                    inv_v[sl].reshape(NVB, P).T.astype(np.float32)
                ),
                "inv_t": inv_t.reshape(1, N).astype(np.float32),
            }
        )
    return run_bass_kernel_spmd(
        nc, in_maps, core_ids=list(range(NCORES)), trace=trace
    )


def kernel(video_embed, text_embed, log_logit_scale, _trace=False, _res_out=None):
    video_embed = np.asarray(video_embed)
    text_embed = np.asarray(text_embed)
    scale = float(np.exp(np.float64(np.asarray(log_logit_scale))))

    v64 = video_embed.astype(np.float64)
    t64 = text_embed.astype(np.float64)
    vn = np.linalg.norm(v64, axis=1)
    tn = np.linalg.norm(t64, axis=1)
    # split the logit scale geometrically between the operands so both sit
    # in the middle of fp8e4m3's dynamic range
    s_half = np.sqrt(scale)
    inv_v = s_half / vn
    inv_t = s_half / tn

    tT32 = np.ascontiguousarray(text_embed.astype(np.float32).T)
    res = _run_device(
        video_embed.astype(np.float32), tT32, inv_v, inv_t, trace=_trace
    )
    if _res_out is not None:
        _res_out.append(res)

    row_sum = np.concatenate(
        [res.results[c]["row_sum"].T.reshape(-1) for c in range(NCORES)]
    ).astype(np.float64)
    row_max_exp = np.concatenate(
        [res.results[c]["row_max"].T.reshape(-1) for c in range(NCORES)]
    ).astype(np.float64)
    row_max = np.log(np.maximum(row_max_exp, 1e-300))

    v_hat = v64 / vn[:, None]
    t_hat = t64 / tn[:, None]
    diag = scale * np.einsum("ij,ij->i", v_hat, t_hat)
    S = row_sum.sum()
    loss = (S - diag.sum()) / N

    cand = np.nonzero(diag >= row_max - MARGIN_BAND)[0]
    k = 0
    for i in cand:
        row = scale * (t_hat @ v_hat[i])
        row[i] = diag[i]
        if int(np.argmax(row)) == i:
            k += 1
    acc = 100.0 * k / N

    return np.float32(loss), np.float32(acc)
